# revision 38
# baseline (speedup 1.0000x reference)
"""Trainium2 Bass kernel for 16-head causal RoPE attention (B=1, L=4096, D=1024).

Distribution: tensor-parallel over heads — each of the 8 cores owns 2 heads
(128 q/k/v dims) and computes a partial output projection; the host sums the
8 partial [1024, 4096] outputs and transposes back to [1, 4096, 1024].

v2 changes vs the first working version:
  - bf16 for x/w/v/wo/att/outT and the yT output (halves DMA + ldweights
    cost, enables DVE 2x/4x modes); the score path (qro/kro) stays f32r.
  - global software pipelining: phase1 (qkv+rope) chunk n+2 and the output
    projection of q-chunk qc-1 are emitted as PE "filler" between attention
    groups so the PE never drains while ACT computes exp (PE pstate stays
    at 2.4 GHz).
  - softmax denominator: reciprocal_approx_fast + gpsimd partition
    broadcast instead of DVE reciprocal + DRAM bounce.
"""

import numpy as np

N_HEAD = 16
HEAD_DIM = 64
HIDDEN = 1024
N_CORES = 8
ROPE_BASE = 10000.0

_CACHE = {}

INTERLEAVE = True
DEBUG = False


def _build(L):
    import concourse.bass as bass
    import concourse.tile as tile
    import concourse.mybir as mybir
    from concourse import bacc
    from concourse.masks import make_identity

    F32 = mybir.dt.float32
    F32R = mybir.dt.float32r
    BF16 = mybir.dt.bfloat16
    Exp = mybir.ActivationFunctionType.Exp

    LC = L // 512          # number of 512-wide q chunks
    KVC = L // 128         # number of 128-wide kv chunks
    HC = HIDDEN // 128     # hidden contraction chunks

    nc = bacc.Bacc("TRN2", target_bir_lowering=False, debug=False,
                   num_devices=N_CORES)

    xT_d = nc.dram_tensor("xT", [HIDDEN, L], BF16, kind="ExternalInput")
    wqT_d = nc.dram_tensor("wqT", [HIDDEN, 128], BF16, kind="ExternalInput")
    wkT_d = nc.dram_tensor("wkT", [HIDDEN, 128], BF16, kind="ExternalInput")
    wvT_d = nc.dram_tensor("wvT", [HIDDEN, 128], BF16, kind="ExternalInput")
    woT_d = nc.dram_tensor("woT", [128, HIDDEN], BF16, kind="ExternalInput")
    cosT_d = nc.dram_tensor("cosT", [128, L], F32, kind="ExternalInput")
    sinT_d = nc.dram_tensor("sinT", [128, L], F32, kind="ExternalInput")
    masks_d = nc.dram_tensor("masks", [128, 4 * 512], BF16, kind="ExternalInput")
    pmat_d = nc.dram_tensor("pmat", [128, 128], F32, kind="ExternalInput")
    yT_d = nc.dram_tensor("yT", [HIDDEN, L], BF16, kind="ExternalOutput")
    interleave, debug = INTERLEAVE, DEBUG
    if debug:
        dbgq_d = nc.dram_tensor("dbgq", [128, L], F32, kind="ExternalOutput")
        dbgk_d = nc.dram_tensor("dbgk", [128, L], F32, kind="ExternalOutput")
        dbgv_d = nc.dram_tensor("dbgv", [128, (L // 128) * 130], BF16,
                                kind="ExternalOutput")
        dbgo_d = nc.dram_tensor("dbgo", [128, L], F32, kind="ExternalOutput")
        dbgi_d = nc.dram_tensor("dbgi", [128, L], F32, kind="ExternalOutput")

    with tile.TileContext(nc) as tc:
        with tc.tile_pool(name="big", bufs=1) as big, \
             tc.tile_pool(name="vno_p", bufs=1) as vno_p, \
             tc.tile_pool(name="w_p", bufs=1) as w_p, \
             tc.tile_pool(name="sm", bufs=3) as sm, \
             tc.tile_pool(name="cssn", bufs=2) as cssn, \
             tc.tile_pool(name="xt_p", bufs=2) as xt_p, \
             tc.tile_pool(name="att_p", bufs=6) as att_p, \
             tc.tile_pool(name="ou_p", bufs=4) as ou_p, \
             tc.tile_pool(name="nrm", bufs=2) as nrm, \
             tc.tile_pool(name="ps_m", bufs=2, space="PSUM") as ps_m, \
             tc.tile_pool(name="ps_st", bufs=2, space="PSUM") as ps_st, \
             tc.tile_pool(name="ps_av", bufs=2, space="PSUM") as ps_av:

            # ---- constants / weights ----
            wq_sb = w_p.tile([128, HC, 128], BF16, tag="wq")
            wk_sb = w_p.tile([128, HC, 128], BF16, tag="wk")
            wv_sb = w_p.tile([128, HC, 128], BF16, tag="wv")
            for w_sb, w_d, q in ((wq_sb, wqT_d, nc.sync),
                                 (wk_sb, wkT_d, nc.gpsimd),
                                 (wv_sb, wvT_d, nc.gpsimd)):
                q.dma_start(
                    out=w_sb,
                    in_=w_d.ap().rearrange("(c p) m -> p c m", p=128))
            wo_sb = w_p.tile([128, HIDDEN], BF16, tag="wo")
            masks_sb = w_p.tile([128, 4, 512], BF16, tag="masks")
            pmat_sb = w_p.tile([128, 128], F32R, tag="pmat")
            nc.sync.dma_start(out=pmat_sb, in_=pmat_d.ap().bitcast(F32R))
            ident = w_p.tile([128, 128], BF16, tag="ident")
            make_identity(nc, ident)

            # v_nat with interleaved ones columns: [kv_in_chunk, kvc, 130]
            # cols 0:64 head0 dims, 64 = 1.0, 65:129 head1 dims, 129 = 1.0
            vno = vno_p.tile([128, KVC, 130], BF16, tag="vno")
            nc.vector.memset(vno[:, :, 64:65], 1.0)
            nc.vector.memset(vno[:, :, 129:130], 1.0)

            qro = big.tile([128, L], F32R, tag="qro")
            kro = big.tile([128, L], F32R, tag="kro")

            # ---- phase1 (qkv projections + rope + v transpose), unitized ----
            def p1_units(n):
                ns = slice(n * 512, (n + 1) * 512)
                ctx = {}

                def u_dma():
                    xt = xt_p.tile([128, HC, 512], BF16, tag="xt")
                    half = HC // 2
                    for hlf in range(2):
                        ks = slice(hlf * half, (hlf + 1) * half)
                        nc.sync.dma_start(
                            out=xt[:, ks, :],
                            in_=xT_d.ap()[hlf * half * 128:(hlf + 1) * half * 128,
                                          ns]
                                .rearrange("(c p) m -> p c m", p=128))
                    cs = cssn.tile([128, 512], F32, tag="cs")
                    sn = cssn.tile([128, 512], F32, tag="sn")
                    nc.gpsimd.dma_start(out=cs, in_=cosT_d.ap()[:, ns])
                    nc.gpsimd.dma_start(out=sn, in_=sinT_d.ap()[:, ns])
                    ctx["xt"], ctx["cs"], ctx["sn"] = xt, cs, sn

                def accum(w_sb, dst_tag, dt):
                    ps = ps_m.tile([128, 512], F32, tag="m")
                    for k in range(HC):
                        nc.tensor.matmul(ps, w_sb[:, k, :], ctx["xt"][:, k, :],
                                         start=(k == 0), stop=(k == HC - 1))
                    t = sm.tile([128, 512], dt, tag=dst_tag)
                    nc.vector.tensor_copy(t, ps)
                    return t

                def rope(t_sb, ro):
                    sw = ps_m.tile([128, 512], F32, tag="m")
                    nc.tensor.matmul(sw, pmat_sb, t_sb)
                    t1 = sm.tile([128, 512], F32, tag="t1")
                    t2 = sm.tile([128, 512], F32, tag="t2")
                    nc.vector.tensor_mul(t1, t_sb.bitcast(F32), ctx["cs"])
                    nc.vector.tensor_mul(t2, sw, ctx["sn"])
                    nc.vector.tensor_add(ro[:, ns], t1, t2)

                def u_q():
                    ctx["qt"] = accum(wq_sb, "qt", F32R)

                def u_rq():
                    rope(ctx["qt"], qro)

                def u_k():
                    ctx["kt"] = accum(wk_sb, "kt", F32R)

                def u_rk():
                    rope(ctx["kt"], kro)

                def u_v():
                    ctx["vt"] = accum(wv_sb, "vt", BF16)

                def u_tr():
                    for j in range(4):
                        kc = n * 4 + j
                        tr = ps_m.tile([128, 128], BF16, tag="m")
                        nc.tensor.transpose(
                            tr, ctx["vt"][:, j * 128:(j + 1) * 128], ident)
                        nc.vector.tensor_copy(vno[:, kc, 0:64], tr[:, 0:64])
                        nc.vector.tensor_copy(vno[:, kc, 65:129], tr[:, 64:128])

                return [u_dma, u_q, u_rq, u_k, u_rk, u_v, u_tr]

            # ---- FIFO filler machinery ----
            work = []

            def filler(k=1):
                for _ in range(k):
                    if not work:
                        return
                    work.pop(0)()

            # ---- attention helpers (512-wide q chunks) ----
            def width(qc, kc):
                di = kc - 4 * qc
                if di <= 0:
                    return 512
                return 384 if di == 1 else 256

            def st_exp(qc, h, g):
                n_g = 2 * (qc + 1)
                hs = h * 64
                stp = ps_st.tile([128, 2, 512], F32, tag="st")
                for j in range(2):
                    kc = 2 * g + j
                    o = 512 - width(qc, kc)
                    nc.tensor.matmul(
                        stp[:, j, o:],
                        kro[hs:hs + 64, kc * 128:(kc + 1) * 128],
                        qro[hs:hs + 64, qc * 512 + o:(qc + 1) * 512])
                att = att_p.tile([128, 2, 512], BF16, tag="att")
                o1 = 512 - width(qc, 2 * g)  # = 0 unless both chunks narrow
                nc.scalar.activation(att[:, :, o1:], stp[:, :, o1:], Exp)
                if g >= n_g - 2:  # diagonal band: apply causal masks
                    for j in range(2):
                        kc = 2 * g + j
                        mj = kc - 4 * qc
                        if 0 <= mj < 4:
                            o = 512 - width(qc, kc)
                            nc.vector.tensor_mul(
                                att[:, j, o:], att[:, j, o:],
                                masks_sb[:, mj, o:])
                return att

            def av_mms(qc, av, h, g_prev, att_prev):
                n_kc = 4 * (qc + 1)
                for j in range(2):
                    kc = 2 * g_prev + j
                    o = 512 - width(qc, kc)
                    nc.tensor.matmul(
                        av[:, o:], vno[:, kc, h * 65:h * 65 + 65],
                        att_prev[:, j, o:],
                        start=(kc == 0), stop=(kc == n_kc - 1),
                        skip_group_check=(o != 0))

            def drain_head(qc, av, h, oun, inv_b):
                # den row lives at psum partition 64 of av
                nc.vector.tensor_copy(
                    oun[h * 64:(h + 1) * 64, :], av[0:64, :])
                den = nrm.tile([1, 512], F32, tag=f"den{h}")
                nc.vector.tensor_copy(den, av[64:65, :])
                inv = nrm.tile([1, 512], F32, tag=f"inv{h}")
                nc.vector.reciprocal_approx_fast(out=inv, in_=den)
                if h == 0:
                    nc.gpsimd.partition_broadcast(
                        inv_b[0:64, :], inv, channels=64)
                else:
                    # gpsimd broadcast mishandles nonzero output partition
                    # offsets; bounce through partition 0
                    tmp = nrm.tile([64, 512], F32, tag="btmp")
                    nc.gpsimd.partition_broadcast(tmp, inv, channels=64)
                    nc.vector.tensor_copy(inv_b[64:128, :], tmp)

            def norm_out(qc, oun, inv_b):
                if debug:
                    qs = slice(qc * 512, (qc + 1) * 512)
                    nc.sync.dma_start(out=dbgo_d.ap()[:, qs], in_=oun)
                    nc.sync.dma_start(out=dbgi_d.ap()[:, qs], in_=inv_b)
                outT = nrm.tile([128, 512], BF16, tag="outT")
                nc.vector.tensor_mul(outT, oun, inv_b)
                return outT

            def attention(qc):
                n_g = 2 * (qc + 1)
                avs = [ps_av.tile([65, 512], F32, tag="av", name=f"av{h}")
                       for h in range(2)]
                pend = [[], []]
                for g in range(n_g):
                    for h in range(2):
                        att = st_exp(qc, h, g)
                        filler()
                        pend[h].append((g, att))
                        if len(pend[h]) > 2:
                            av_mms(qc, avs[h], h, *pend[h].pop(0))
                            filler()
                for h in range(2):
                    while pend[h]:
                        av_mms(qc, avs[h], h, *pend[h].pop(0))
                        filler()
                oun = ou_p.tile([128, 512], F32, tag="ou")
                inv_b = nrm.tile([128, 512], F32, tag="invb")
                for h in range(2):
                    drain_head(qc, avs[h], h, oun, inv_b)
                return norm_out(qc, oun, inv_b)

            def attention2(qa, qb):
                """Two q-chunks interleaved, one head at a time (the two
                streams hide each other's exp latency; psum stays in 8
                banks because only one head's accumulators live per pass)."""
                st8 = {}
                for qc in (qa, qb):
                    oun = ou_p.tile([128, 512], F32, tag="ou")
                    inv_b = nrm.tile([128, 512], F32, tag="invb")
                    st8[qc] = (oun, inv_b)
                for h in range(2):
                    avx = {qc: ps_av.tile([65, 512], F32, tag="av",
                                          name=f"av{qc}")
                           for qc in (qa, qb)}
                    pend = {qa: [], qb: []}
                    for g in range(max(2 * (qa + 1), 2 * (qb + 1))):
                        for qc in (qa, qb):
                            if g >= 2 * (qc + 1):
                                continue
                            att = st_exp(qc, h, g)
                            filler()
                            pend[qc].append((g, att))
                            if len(pend[qc]) > 2:
                                av_mms(qc, avx[qc], h, *pend[qc].pop(0))
                                filler()
                    for qc in (qa, qb):
                        while pend[qc]:
                            av_mms(qc, avx[qc], h, *pend[qc].pop(0))
                            filler()
                        drain_head(qc, avx[qc], h, *st8[qc])
                return [(qc, norm_out(qc, *st8[qc])) for qc in (qa, qb)]

            # ---- output projection units (deferred into the next q chunk) --
            def oproj_units(qc, outT):
                qs = slice(qc * 512, (qc + 1) * 512)
                units = []

                def mk(e):
                    def u():
                        ps_y = ps_m.tile([128, 512], F32, tag="m")
                        nc.tensor.matmul(
                            ps_y, wo_sb[:, e * 128:(e + 1) * 128], outT)
                        y_sb = nrm.tile([128, 512], BF16, tag=f"y{e % 2}")
                        nc.vector.tensor_copy(y_sb, ps_y)
                        nc.sync.dma_start(
                            out=yT_d.ap()[e * 128:(e + 1) * 128, qs], in_=y_sb)
                    return u

                for e in range(HC):
                    units.append(mk(e))
                return units

            # ---- schedule ----
            for u in p1_units(0):
                u()
            # deferred bulk constant loads (needed only from attention onward)
            nc.gpsimd.dma_start(out=wo_sb, in_=woT_d.ap())
            nc.gpsimd.dma_start(
                out=masks_sb, in_=masks_d.ap().rearrange("p (j n) -> p j n", j=4))
            if LC > 1:
                for u in p1_units(1):
                    u()
            if interleave:
                for qc in range(max(LC - 2, 0)):
                    if qc + 2 < LC:
                        work.extend(p1_units(qc + 2))
                    outT = attention(qc)
                    work.extend(oproj_units(qc, outT))
                if LC >= 2:
                    for qcT, outT in attention2(LC - 2, LC - 1):
                        work.extend(oproj_units(qcT, outT))
                while work:
                    work.pop(0)()
            else:
                for n in range(2, LC):
                    for u in p1_units(n):
                        u()
                for qc in range(LC):
                    outT = attention(qc)
                    work.extend(oproj_units(qc, outT))
                    while work:
                        work.pop(0)()
            if debug:
                nc.sync.dma_start(out=dbgq_d.ap().bitcast(F32R), in_=qro)
                nc.sync.dma_start(out=dbgk_d.ap().bitcast(F32R), in_=kro)
                nc.sync.dma_start(
                    out=dbgv_d.ap().rearrange("p (c m) -> p c m", m=130),
                    in_=vno)

    nc.compile()
    return nc


def _host_prep(x, wq, wk, wv, wo, L):
    """Build per-core input maps (numpy only)."""
    import ml_dtypes
    BF = ml_dtypes.bfloat16

    x2 = np.ascontiguousarray(x.reshape(L, HIDDEN))
    xT = np.ascontiguousarray(x2.T.astype(BF))

    # rope tables, transposed + duplicated for the two heads on each core
    inv_freq = 1.0 / (ROPE_BASE ** (np.arange(0, HEAD_DIM, 2, dtype=np.float64)
                                    / HEAD_DIM))
    freqs = np.arange(L, dtype=np.float64)[:, None] * inv_freq[None, :]
    emb = np.concatenate([freqs, freqs], axis=-1)          # [L, 64]
    cosT = np.cos(emb).T.astype(np.float32)                # [64, L]
    sinT = np.sin(emb).T.astype(np.float32)
    cosT2 = np.ascontiguousarray(np.concatenate([cosT, cosT], axis=0))
    sinT2 = np.ascontiguousarray(np.concatenate([sinT, sinT], axis=0))

    # causal masks for the 4 diagonal kv chunks of each 512-q chunk
    kv = np.arange(128)[:, None]
    q = np.arange(512)[None, :]
    masks = np.concatenate(
        [(q >= j * 128 + kv).astype(BF) for j in range(4)], axis=1)
    masks = np.ascontiguousarray(masks)                    # [128, 2048] bf16

    # rotate-half permutation (as matmul lhsT), block-diag for 2 heads
    P = np.zeros((64, 64), np.float32)
    P[np.arange(32) + 32, np.arange(32)] = -1.0
    P[np.arange(32), np.arange(32) + 32] = 1.0
    pmat = np.zeros((128, 128), np.float32)
    pmat[0:64, 0:64] = P
    pmat[64:128, 64:128] = P

    in_maps = []
    for c in range(N_CORES):
        rows = slice(c * 128, (c + 1) * 128)
        in_maps.append({
            "xT": xT,
            "wqT": np.ascontiguousarray(
                (wq[rows, :].T * np.float32(1.0 / 8.0)).astype(BF)),
            "wkT": np.ascontiguousarray(wk[rows, :].T.astype(BF)),
            "wvT": np.ascontiguousarray(wv[rows, :].T.astype(BF)),
            "woT": np.ascontiguousarray(wo[:, rows].T.astype(BF)),
            "cosT": cosT2,
            "sinT": sinT2,
            "masks": masks,
            "pmat": pmat,
        })
    return in_maps


def _ensure_profile_hook():
    """The agent image's antenv lacks axon_hooks; recreate it from the boot
    package so trace=True can capture NTFF profiles."""
    import sys, types
    try:
        from antenv.axon_hooks import get_axon_ntff_profile_hook  # noqa: F401
        return
    except ImportError:
        pass
    try:
        from trn_agent_boot.trn_boot import _ntff_profile_via_ctypes
        hook = _ntff_profile_via_ctypes('/opt/axon/libaxon_pjrt.so')
    except Exception:
        hook = None
    mod = types.ModuleType("antenv.axon_hooks")
    mod.get_axon_ntff_profile_hook = lambda: hook
    mod.set_axon_ntff_profile_hook = lambda h: None
    sys.modules["antenv.axon_hooks"] = mod


def _run(x, wq, wk, wv, wo, trace=False, trace_cores=None):
    from concourse.bass_utils import run_bass_kernel_spmd

    if trace:
        _ensure_profile_hook()

    B, L, D = x.shape
    assert (B, D) == (1, HIDDEN)
    if L not in _CACHE:
        _CACHE[L] = _build(L)
    nc = _CACHE[L]
    in_maps = _host_prep(np.asarray(x, np.float32), wq, wk, wv, wo, L)
    res = run_bass_kernel_spmd(
        nc, in_maps, core_ids=list(range(N_CORES)),
        trace=trace, trace_cores=trace_cores)
    acc = np.zeros((HIDDEN, L), np.float64)
    for r in res.results:
        acc += r["yT"].astype(np.float64)
    y = np.ascontiguousarray(acc.T.astype(np.float32)).reshape(1, L, HIDDEN)
    return y, res


def kernel(x, wq, wk, wv, wo):
    y, _ = _run(np.asarray(x), np.asarray(wq), np.asarray(wk),
                np.asarray(wv), np.asarray(wo))
    return y


# revision 39
# speedup vs baseline: 1.2055x; 1.2055x over previous
"""Trainium2 Bass kernel for 16-head causal RoPE attention (B=1, L=4096, D=1024).

Distribution: tensor-parallel over heads — each of the 8 cores owns 2 heads
(128 q/k/v dims) and computes a partial output projection; the host sums the
8 partial [1024, 4096] outputs and transposes back to [1, 4096, 1024].

v2 changes vs the first working version:
  - bf16 for x/w/v/wo/att/outT and the yT output (halves DMA + ldweights
    cost, enables DVE 2x/4x modes); the score path (qro/kro) stays f32r.
  - global software pipelining: phase1 (qkv+rope) chunk n+2 and the output
    projection of q-chunk qc-1 are emitted as PE "filler" between attention
    groups so the PE never drains while ACT computes exp (PE pstate stays
    at 2.4 GHz).
  - softmax denominator: reciprocal_approx_fast + gpsimd partition
    broadcast instead of DVE reciprocal + DRAM bounce.
"""

import numpy as np

N_HEAD = 16
HEAD_DIM = 64
HIDDEN = 1024
N_CORES = 8
ROPE_BASE = 10000.0

_CACHE = {}

INTERLEAVE = True
DEBUG = False


def _build(L):
    import concourse.bass as bass
    import concourse.tile as tile
    import concourse.mybir as mybir
    from concourse import bacc
    from concourse.masks import make_identity

    F32 = mybir.dt.float32
    F32R = mybir.dt.float32r
    BF16 = mybir.dt.bfloat16
    Exp = mybir.ActivationFunctionType.Exp

    LC = L // 512          # number of 512-wide q chunks
    KVC = L // 128         # number of 128-wide kv chunks
    HC = HIDDEN // 128     # hidden contraction chunks

    nc = bacc.Bacc("TRN2", target_bir_lowering=False, debug=False,
                   num_devices=N_CORES)

    xT_d = nc.dram_tensor("xT", [HIDDEN, L], BF16, kind="ExternalInput")
    wqT_d = nc.dram_tensor("wqT", [HIDDEN, 128], BF16, kind="ExternalInput")
    wkT_d = nc.dram_tensor("wkT", [HIDDEN, 128], BF16, kind="ExternalInput")
    wvT_d = nc.dram_tensor("wvT", [HIDDEN, 128], BF16, kind="ExternalInput")
    woT_d = nc.dram_tensor("woT", [128, HIDDEN], BF16, kind="ExternalInput")
    cosT_d = nc.dram_tensor("cosT", [128, L], F32, kind="ExternalInput")
    sinT_d = nc.dram_tensor("sinT", [128, L], F32, kind="ExternalInput")
    masks_d = nc.dram_tensor("masks", [128, 4 * 512], BF16, kind="ExternalInput")
    pmat_d = nc.dram_tensor("pmat", [128, 128], F32, kind="ExternalInput")
    yT_d = nc.dram_tensor("yT", [HIDDEN, L], BF16, kind="ExternalOutput")
    interleave, debug = INTERLEAVE, DEBUG
    if debug:
        dbgq_d = nc.dram_tensor("dbgq", [128, L], F32, kind="ExternalOutput")
        dbgk_d = nc.dram_tensor("dbgk", [128, L], F32, kind="ExternalOutput")
        dbgv_d = nc.dram_tensor("dbgv", [128, (L // 128) * 130], BF16,
                                kind="ExternalOutput")
        dbgo_d = nc.dram_tensor("dbgo", [128, L], F32, kind="ExternalOutput")
        dbgi_d = nc.dram_tensor("dbgi", [128, L], F32, kind="ExternalOutput")

    with tile.TileContext(nc) as tc:
        with tc.tile_pool(name="big", bufs=1) as big, \
             tc.tile_pool(name="vno_p", bufs=1) as vno_p, \
             tc.tile_pool(name="w_p", bufs=1) as w_p, \
             tc.tile_pool(name="sm", bufs=3) as sm, \
             tc.tile_pool(name="cssn", bufs=2) as cssn, \
             tc.tile_pool(name="xt_p", bufs=2) as xt_p, \
             tc.tile_pool(name="att_p", bufs=6) as att_p, \
             tc.tile_pool(name="ou_p", bufs=4) as ou_p, \
             tc.tile_pool(name="nrm", bufs=2) as nrm, \
             tc.tile_pool(name="ps_m", bufs=2, space="PSUM") as ps_m, \
             tc.tile_pool(name="ps_st", bufs=2, space="PSUM") as ps_st, \
             tc.tile_pool(name="ps_av", bufs=2, space="PSUM") as ps_av:

            # ---- constants / weights ----
            wq_sb = w_p.tile([128, HC, 128], BF16, tag="wq")
            wk_sb = w_p.tile([128, HC, 128], BF16, tag="wk")
            wv_sb = w_p.tile([128, HC, 128], BF16, tag="wv")
            for w_sb, w_d, q in ((wq_sb, wqT_d, nc.sync),
                                 (wk_sb, wkT_d, nc.gpsimd),
                                 (wv_sb, wvT_d, nc.gpsimd)):
                q.dma_start(
                    out=w_sb,
                    in_=w_d.ap().rearrange("(c p) m -> p c m", p=128))
            wo_sb = w_p.tile([128, HIDDEN], BF16, tag="wo")
            masks_sb = w_p.tile([128, 4, 512], BF16, tag="masks")
            pmat_sb = w_p.tile([128, 128], F32R, tag="pmat")
            nc.sync.dma_start(out=pmat_sb, in_=pmat_d.ap().bitcast(F32R))
            ident = w_p.tile([128, 128], BF16, tag="ident")
            make_identity(nc, ident)

            # v_nat with interleaved ones columns: [kv_in_chunk, kvc, 130]
            # cols 0:64 head0 dims, 64 = 1.0, 65:129 head1 dims, 129 = 1.0
            vno = vno_p.tile([128, KVC, 130], BF16, tag="vno")
            nc.vector.memset(vno[:, :, 64:65], 1.0)
            nc.vector.memset(vno[:, :, 129:130], 1.0)

            qro = big.tile([128, L], F32R, tag="qro")
            kro = big.tile([128, L], F32R, tag="kro")

            # ---- phase1 (qkv projections + rope + v transpose), unitized ----
            def p1_units(n):
                ns = slice(n * 512, (n + 1) * 512)
                ctx = {}

                def u_dma():
                    xt = xt_p.tile([128, HC, 512], BF16, tag="xt")
                    half = HC // 2
                    for hlf in range(2):
                        ks = slice(hlf * half, (hlf + 1) * half)
                        nc.sync.dma_start(
                            out=xt[:, ks, :],
                            in_=xT_d.ap()[hlf * half * 128:(hlf + 1) * half * 128,
                                          ns]
                                .rearrange("(c p) m -> p c m", p=128))
                    cs = cssn.tile([128, 512], F32, tag="cs")
                    sn = cssn.tile([128, 512], F32, tag="sn")
                    nc.sync.dma_start(out=cs, in_=cosT_d.ap()[:, ns])
                    nc.sync.dma_start(out=sn, in_=sinT_d.ap()[:, ns])
                    ctx["xt"], ctx["cs"], ctx["sn"] = xt, cs, sn

                def accum(w_sb, dst_tag, dt):
                    ps = ps_m.tile([128, 512], F32, tag="m")
                    for k in range(HC):
                        nc.tensor.matmul(ps, w_sb[:, k, :], ctx["xt"][:, k, :],
                                         start=(k == 0), stop=(k == HC - 1))
                    t = sm.tile([128, 512], dt, tag=dst_tag)
                    nc.vector.tensor_copy(t, ps)
                    return t

                def rope(t_sb, ro):
                    sw = ps_m.tile([128, 512], F32, tag="m")
                    nc.tensor.matmul(sw, pmat_sb, t_sb)
                    t1 = sm.tile([128, 512], F32, tag="t1")
                    t2 = sm.tile([128, 512], F32, tag="t2")
                    nc.vector.tensor_mul(t1, t_sb.bitcast(F32), ctx["cs"])
                    nc.vector.tensor_mul(t2, sw, ctx["sn"])
                    nc.vector.tensor_add(ro[:, ns], t1, t2)

                def u_q():
                    ctx["qt"] = accum(wq_sb, "qt", F32R)

                def u_rq():
                    rope(ctx["qt"], qro)

                def u_k():
                    ctx["kt"] = accum(wk_sb, "kt", F32R)

                def u_rk():
                    rope(ctx["kt"], kro)

                def u_v():
                    ctx["vt"] = accum(wv_sb, "vt", BF16)

                def u_tr():
                    for j in range(4):
                        kc = n * 4 + j
                        tr = ps_m.tile([128, 128], BF16, tag="m")
                        nc.tensor.transpose(
                            tr, ctx["vt"][:, j * 128:(j + 1) * 128], ident)
                        nc.vector.tensor_copy(vno[:, kc, 0:64], tr[:, 0:64])
                        nc.vector.tensor_copy(vno[:, kc, 65:129], tr[:, 64:128])

                return [u_dma, u_q, u_rq, u_k, u_rk, u_v, u_tr]

            # ---- FIFO filler machinery ----
            work = []

            def filler(k=1):
                for _ in range(k):
                    if not work:
                        return
                    work.pop(0)()

            # ---- attention helpers (512-wide q chunks) ----
            def width(qc, kc):
                di = kc - 4 * qc
                if di <= 0:
                    return 512
                return 384 if di == 1 else 256

            def st_exp(qc, h, g):
                n_g = 2 * (qc + 1)
                hs = h * 64
                stp = ps_st.tile([128, 2, 512], F32, tag="st")
                for j in range(2):
                    kc = 2 * g + j
                    o = 512 - width(qc, kc)
                    nc.tensor.matmul(
                        stp[:, j, o:],
                        kro[hs:hs + 64, kc * 128:(kc + 1) * 128],
                        qro[hs:hs + 64, qc * 512 + o:(qc + 1) * 512])
                att = att_p.tile([128, 2, 512], BF16, tag="att")
                o1 = 512 - width(qc, 2 * g)  # = 0 unless both chunks narrow
                nc.scalar.activation(att[:, :, o1:], stp[:, :, o1:], Exp)
                if g >= n_g - 2:  # diagonal band: apply causal masks
                    for j in range(2):
                        kc = 2 * g + j
                        mj = kc - 4 * qc
                        if 0 <= mj < 4:
                            o = 512 - width(qc, kc)
                            nc.vector.tensor_mul(
                                att[:, j, o:], att[:, j, o:],
                                masks_sb[:, mj, o:])
                return att

            def av_mms(qc, av, h, g_prev, att_prev):
                n_kc = 4 * (qc + 1)
                for j in range(2):
                    kc = 2 * g_prev + j
                    o = 512 - width(qc, kc)
                    nc.tensor.matmul(
                        av[:, o:], vno[:, kc, h * 65:h * 65 + 65],
                        att_prev[:, j, o:],
                        start=(kc == 0), stop=(kc == n_kc - 1),
                        skip_group_check=(o != 0))

            def drain_head(qc, av, h, oun, inv_b):
                # den row lives at psum partition 64 of av
                nc.vector.tensor_copy(
                    oun[h * 64:(h + 1) * 64, :], av[0:64, :])
                den = nrm.tile([1, 512], F32, tag=f"den{h}")
                nc.vector.tensor_copy(den, av[64:65, :])
                inv = nrm.tile([1, 512], F32, tag=f"inv{h}")
                nc.vector.reciprocal_approx_fast(out=inv, in_=den)
                if h == 0:
                    nc.gpsimd.partition_broadcast(
                        inv_b[0:64, :], inv, channels=64)
                else:
                    # gpsimd broadcast mishandles nonzero output partition
                    # offsets; bounce through partition 0
                    tmp = nrm.tile([64, 512], F32, tag="btmp")
                    nc.gpsimd.partition_broadcast(tmp, inv, channels=64)
                    nc.vector.tensor_copy(inv_b[64:128, :], tmp)

            def norm_out(qc, oun, inv_b):
                if debug:
                    qs = slice(qc * 512, (qc + 1) * 512)
                    nc.sync.dma_start(out=dbgo_d.ap()[:, qs], in_=oun)
                    nc.sync.dma_start(out=dbgi_d.ap()[:, qs], in_=inv_b)
                outT = nrm.tile([128, 512], BF16, tag="outT")
                nc.vector.tensor_mul(outT, oun, inv_b)
                return outT

            def attention(qc):
                n_g = 2 * (qc + 1)
                avs = [ps_av.tile([65, 512], F32, tag="av", name=f"av{h}")
                       for h in range(2)]
                pend = [[], []]
                for g in range(n_g):
                    for h in range(2):
                        att = st_exp(qc, h, g)
                        filler()
                        pend[h].append((g, att))
                        if len(pend[h]) > 2:
                            av_mms(qc, avs[h], h, *pend[h].pop(0))
                            filler()
                for h in range(2):
                    while pend[h]:
                        av_mms(qc, avs[h], h, *pend[h].pop(0))
                        filler()
                oun = ou_p.tile([128, 512], F32, tag="ou")
                inv_b = nrm.tile([128, 512], F32, tag="invb")
                for h in range(2):
                    drain_head(qc, avs[h], h, oun, inv_b)
                return norm_out(qc, oun, inv_b)

            def attention2(qa, qb):
                """Two q-chunks interleaved, one head at a time (the two
                streams hide each other's exp latency; psum stays in 8
                banks because only one head's accumulators live per pass)."""
                st8 = {}
                for qc in (qa, qb):
                    oun = ou_p.tile([128, 512], F32, tag="ou")
                    inv_b = nrm.tile([128, 512], F32, tag="invb")
                    st8[qc] = (oun, inv_b)
                for h in range(2):
                    avx = {qc: ps_av.tile([65, 512], F32, tag="av",
                                          name=f"av{qc}")
                           for qc in (qa, qb)}
                    pend = {qa: [], qb: []}
                    for g in range(max(2 * (qa + 1), 2 * (qb + 1))):
                        for qc in (qa, qb):
                            if g >= 2 * (qc + 1):
                                continue
                            att = st_exp(qc, h, g)
                            filler()
                            pend[qc].append((g, att))
                            if len(pend[qc]) > 2:
                                av_mms(qc, avx[qc], h, *pend[qc].pop(0))
                                filler()
                    for qc in (qa, qb):
                        while pend[qc]:
                            av_mms(qc, avx[qc], h, *pend[qc].pop(0))
                            filler()
                        drain_head(qc, avx[qc], h, *st8[qc])
                return [(qc, norm_out(qc, *st8[qc])) for qc in (qa, qb)]

            # ---- output projection units (deferred into the next q chunk) --
            def oproj_units(qc, outT):
                qs = slice(qc * 512, (qc + 1) * 512)
                units = []

                def mk(e):
                    def u():
                        ps_y = ps_m.tile([128, 512], F32, tag="m")
                        nc.tensor.matmul(
                            ps_y, wo_sb[:, e * 128:(e + 1) * 128], outT)
                        y_sb = nrm.tile([128, 512], BF16, tag=f"y{e % 2}")
                        nc.vector.tensor_copy(y_sb, ps_y)
                        nc.sync.dma_start(
                            out=yT_d.ap()[e * 128:(e + 1) * 128, qs], in_=y_sb)
                    return u

                for e in range(HC):
                    units.append(mk(e))
                return units

            # ---- schedule ----
            for u in p1_units(0):
                u()
            # deferred bulk constant loads (needed only from attention onward)
            nc.gpsimd.dma_start(out=wo_sb, in_=woT_d.ap())
            nc.gpsimd.dma_start(
                out=masks_sb, in_=masks_d.ap().rearrange("p (j n) -> p j n", j=4))
            if LC > 1:
                for u in p1_units(1):
                    u()
            if interleave:
                for qc in range(max(LC - 2, 0)):
                    if qc + 2 < LC:
                        work.extend(p1_units(qc + 2))
                    outT = attention(qc)
                    work.extend(oproj_units(qc, outT))
                if LC >= 2:
                    for qcT, outT in attention2(LC - 2, LC - 1):
                        work.extend(oproj_units(qcT, outT))
                while work:
                    work.pop(0)()
            else:
                for n in range(2, LC):
                    for u in p1_units(n):
                        u()
                for qc in range(LC):
                    outT = attention(qc)
                    work.extend(oproj_units(qc, outT))
                    while work:
                        work.pop(0)()
            if debug:
                nc.sync.dma_start(out=dbgq_d.ap().bitcast(F32R), in_=qro)
                nc.sync.dma_start(out=dbgk_d.ap().bitcast(F32R), in_=kro)
                nc.sync.dma_start(
                    out=dbgv_d.ap().rearrange("p (c m) -> p c m", m=130),
                    in_=vno)

    nc.compile()
    return nc


def _host_prep(x, wq, wk, wv, wo, L):
    """Build per-core input maps (numpy only)."""
    import ml_dtypes
    BF = ml_dtypes.bfloat16

    x2 = np.ascontiguousarray(x.reshape(L, HIDDEN))
    xT = np.ascontiguousarray(x2.T.astype(BF))

    # rope tables, transposed + duplicated for the two heads on each core
    inv_freq = 1.0 / (ROPE_BASE ** (np.arange(0, HEAD_DIM, 2, dtype=np.float64)
                                    / HEAD_DIM))
    freqs = np.arange(L, dtype=np.float64)[:, None] * inv_freq[None, :]
    emb = np.concatenate([freqs, freqs], axis=-1)          # [L, 64]
    cosT = np.cos(emb).T.astype(np.float32)                # [64, L]
    sinT = np.sin(emb).T.astype(np.float32)
    cosT2 = np.ascontiguousarray(np.concatenate([cosT, cosT], axis=0))
    sinT2 = np.ascontiguousarray(np.concatenate([sinT, sinT], axis=0))

    # causal masks for the 4 diagonal kv chunks of each 512-q chunk
    kv = np.arange(128)[:, None]
    q = np.arange(512)[None, :]
    masks = np.concatenate(
        [(q >= j * 128 + kv).astype(BF) for j in range(4)], axis=1)
    masks = np.ascontiguousarray(masks)                    # [128, 2048] bf16

    # rotate-half permutation (as matmul lhsT), block-diag for 2 heads
    P = np.zeros((64, 64), np.float32)
    P[np.arange(32) + 32, np.arange(32)] = -1.0
    P[np.arange(32), np.arange(32) + 32] = 1.0
    pmat = np.zeros((128, 128), np.float32)
    pmat[0:64, 0:64] = P
    pmat[64:128, 64:128] = P

    in_maps = []
    for c in range(N_CORES):
        rows = slice(c * 128, (c + 1) * 128)
        in_maps.append({
            "xT": xT,
            "wqT": np.ascontiguousarray(
                (wq[rows, :].T * np.float32(1.0 / 8.0)).astype(BF)),
            "wkT": np.ascontiguousarray(wk[rows, :].T.astype(BF)),
            "wvT": np.ascontiguousarray(wv[rows, :].T.astype(BF)),
            "woT": np.ascontiguousarray(wo[:, rows].T.astype(BF)),
            "cosT": cosT2,
            "sinT": sinT2,
            "masks": masks,
            "pmat": pmat,
        })
    return in_maps


def _ensure_profile_hook():
    """The agent image's antenv lacks axon_hooks; recreate it from the boot
    package so trace=True can capture NTFF profiles."""
    import sys, types
    try:
        from antenv.axon_hooks import get_axon_ntff_profile_hook  # noqa: F401
        return
    except ImportError:
        pass
    try:
        from trn_agent_boot.trn_boot import _ntff_profile_via_ctypes
        hook = _ntff_profile_via_ctypes('/opt/axon/libaxon_pjrt.so')
    except Exception:
        hook = None
    mod = types.ModuleType("antenv.axon_hooks")
    mod.get_axon_ntff_profile_hook = lambda: hook
    mod.set_axon_ntff_profile_hook = lambda h: None
    sys.modules["antenv.axon_hooks"] = mod


def _run(x, wq, wk, wv, wo, trace=False, trace_cores=None):
    from concourse.bass_utils import run_bass_kernel_spmd

    if trace:
        _ensure_profile_hook()

    B, L, D = x.shape
    assert (B, D) == (1, HIDDEN)
    if L not in _CACHE:
        _CACHE[L] = _build(L)
    nc = _CACHE[L]
    in_maps = _host_prep(np.asarray(x, np.float32), wq, wk, wv, wo, L)
    res = run_bass_kernel_spmd(
        nc, in_maps, core_ids=list(range(N_CORES)),
        trace=trace, trace_cores=trace_cores)
    acc = np.zeros((HIDDEN, L), np.float64)
    for r in res.results:
        acc += r["yT"].astype(np.float64)
    y = np.ascontiguousarray(acc.T.astype(np.float32)).reshape(1, L, HIDDEN)
    return y, res


def kernel(x, wq, wk, wv, wo):
    y, _ = _run(np.asarray(x), np.asarray(wq), np.asarray(wk),
                np.asarray(wv), np.asarray(wo))
    return y
